# revision 33
# baseline (speedup 1.0000x reference)
"""Multi-head SwiGLU feed-forward (nn_MultiHeadFeedForward) Trainium2 kernel.

Math (per head h of 16, head_dim d=128, ffn f=512):
    g = x_h @ gate_w[h]      # [T,128]@[128,512]
    u = x_h @ up_w[h]
    out_h = (silu(g)*u) @ down_w[h]   # [T,512]@[512,128]

Sharding: 2 heads per core across 8 cores (no cross-core communication).
On-chip layout is feature-major ("transposed"): activations live as
[feature, token] tiles so every matmul contracts along the partition dim
without any on-chip transposes.  The host pre-transposes x into
xT[h, d, t] per core and un-transposes the output.

Steady-state budget per 2-tile pair (512 tokens), from the hw cost model:
  PE   16 N=256 gate/up MMs + 4 N=512 down MMs = 2610 ns
  ACT  4 half-silus [128,512] + 0.5 drain copy = 2565 ns
  DVE  2 muls [128,1024] + 0.5 drain copy      = 2579 ns
All three engines sit within 2% of each other, so scheduling latency (not
throughput) sets the ~2850ns/pair effective pace.  Key mechanisms:
  - down-proj pairs pop at even iterations with a 3-tile lag, accumulating
    into the bank-A gate-psum tile of iteration k-1 (1-bank tiles give
    per-bank dependency tracking, and that tile's half-silu retires a full
    iteration before the pop, so the pair matmuls issue back-to-back)
  - per-pair drain: ACT/DVE alternate the PSUM->SBUF bf16 copy 1:1, then
    one DMA per pair (single-semaphore waits; any other copy split pushes
    one engine into the jam and regresses)
  - 8 dummy matmuls on zeroed scratch warm the HAM clock gate during the
    initial DMA wait; first/last DMAs are ordered and partition-split so
    neither the lead-in nor the tail sits on a 9.5us descriptor chain
Measured: 195.3-196.4us/core across runs (pure-PE bf16 floor 163.8us;
session baseline was 216.2us).
"""

import os
import sys

import numpy as np

for _p in ("/opt/trn_rl_repo",):
    if _p not in sys.path and os.path.isdir(_p):
        sys.path.insert(0, _p)

import concourse.bass as bass
import concourse.mybir as mybir
from concourse import bacc
import concourse.tile as tile
from concourse.bass_utils import run_bass_kernel_spmd

B, S, EMB = 4, 4096, 2048
HEADS, HD, FFN = 16, 128, 512
T = B * S                      # 16384 tokens
N_CORES = 8
HPC = HEADS // N_CORES         # heads per core = 2
TOK = 256                      # tokens per on-chip tile
NT = T // TOK                  # token tiles per head
NCH = FFN // HD                # ffn chunks of 128 = 4

F32 = mybir.dt.float32
BF16 = mybir.dt.bfloat16
AF = mybir.ActivationFunctionType


def _build_nc():
    nc = bacc.Bacc("TRN2", target_bir_lowering=False)

    xT = nc.dram_tensor("xT", [HPC, HD, T], BF16, kind="ExternalInput")
    gw = nc.dram_tensor("gw", [HPC, HD, FFN], BF16, kind="ExternalInput")
    uw = nc.dram_tensor("uw", [HPC, HD, FFN], BF16, kind="ExternalInput")
    dw = nc.dram_tensor("dw", [HPC, FFN, HD], BF16, kind="ExternalInput")
    outT = nc.dram_tensor("outT", [HPC, HD, T], BF16, kind="ExternalOutput")

    with tile.TileContext(nc) as tc:
        with (
            tc.tile_pool(name="wpool", bufs=1) as wpool,
            tc.tile_pool(name="gpoolA", bufs=2, space="PSUM") as gpoolA,
            tc.tile_pool(name="gpoolB", bufs=2, space="PSUM") as gpoolB,
            tc.tile_pool(name="upool", bufs=2, space="PSUM") as upool,
            tc.tile_pool(name="sgpool", bufs=6) as sgpool,
            tc.tile_pool(name="hpool", bufs=6) as hpool,
            tc.tile_pool(name="opool", bufs=16) as opool,
        ):
            # weights + the entire x shard resident in SBUF for the kernel
            gw_s = wpool.tile([HD, HPC, FFN], BF16)
            uw_s = wpool.tile([HD, HPC, FFN], BF16)
            dw_s = wpool.tile([HD, HPC, NCH, HD], BF16)
            xs_full = wpool.tile([HD, HPC, T], BF16)
            # PE warmup: dummy matmuls on zeroed scratch (no DMA deps) fill
            # the engine-start -> first-data window so the HAM clock gate
            # un-throttles (needs ~3.4us sustained busy) before real work
            scr = wpool.tile([HD, 512], BF16)
            nc.vector.memset(scr[:], 0)
            wps = gpoolA.tile([HD, 2 * TOK], F32, name="warm", tag="ga")
            for _ in range(8):
                nc.tensor.matmul(
                    wps[:], lhsT=scr[:, 0:HD], rhs=scr[:], start=True, stop=True
                )
            # DMA issue costs ~650ns per dma_start on the sync sequencer, so
            # order matters more than splitting: the first gate matmul needs
            # gw[0] + the first x tokens, so those two calls go first.
            XC = 1024
            nc.sync.dma_start(out=gw_s[:, 0, :], in_=gw[0])
            # small first chunk so tile 0 can start ~4us sooner
            nc.sync.dma_start(out=xs_full[:, 0, 0:256], in_=xT[0, :, 0:256])
            nc.sync.dma_start(out=uw_s[:, 0, :], in_=uw[0])
            nc.sync.dma_start(out=xs_full[:, 0, 256:512], in_=xT[0, :, 256:512])
            nc.sync.dma_start(
                out=dw_s[:, 0, :, :],
                in_=dw[0].rearrange("(c p) d -> p c d", p=HD),
            )
            nc.sync.dma_start(out=xs_full[:, 0, 512:XC], in_=xT[0, :, 512:XC])
            for xc in range(1, T // XC):
                c0 = xc * XC
                nc.sync.dma_start(
                    out=xs_full[:, 0, c0 : c0 + XC], in_=xT[0, :, c0 : c0 + XC]
                )
            for h in range(1, HPC):
                nc.sync.dma_start(out=gw_s[:, h, :], in_=gw[h])
                nc.sync.dma_start(out=uw_s[:, h, :], in_=uw[h])
                nc.sync.dma_start(
                    out=dw_s[:, h, :, :],
                    in_=dw[h].rearrange("(c p) d -> p c d", p=HD),
                )
                # h1 is consumed ~95us in: bigger chunks halve the issue
                # count so the early out-DMAs get sync issue slots sooner
                for xc in range(T // (2 * XC)):
                    c0 = xc * 2 * XC
                    nc.sync.dma_start(
                        out=xs_full[:, h, c0 : c0 + 2 * XC],
                        in_=xT[h, :, c0 : c0 + 2 * XC],
                    )

            tiles = [(h, t) for h in range(HPC) for t in range(NT)]
            K = len(tiles)
            pend = []       # [(hh, h, t), ...] oldest first
            gps_hist = {}   # tile index -> gate psum tile (last 2 kept)
            n_pairs = [0]

            def emit_down_pair(p, ops):
                # down-proj for a PAIR of tiles: 4 matmuls of N=2*TOK reading
                # the pair's joint hh, accumulating into the overlay PSUM
                # region `ops` (bank 0 of a retired gate-psum tile); then one
                # [128, 512] copy into a small out buffer, alternating engine
                # per pair (keeps ACT/DVE evenly loaded and each out-DMA
                # waiting on a single engine), then one DMA per pair.
                phh, ph, pt = p
                for c in range(NCH):
                    nc.tensor.matmul(
                        ops,
                        lhsT=dw_s[:, ph, c, :],
                        rhs=phh[:, c, :],
                        start=(c == 0),
                        stop=(c == NCH - 1),
                    )
                ob = opool.tile([HD, 2 * TOK], BF16, name=f"ob_{ph}_{pt}", tag="ob")
                if n_pairs[0] % 2 == 0:
                    nc.scalar.copy(ob[:], ops)
                else:
                    nc.vector.tensor_copy(ob[:], ops)
                n_pairs[0] += 1
                pt0 = (pt - 1) * TOK
                if n_pairs[0] > 62:
                    # tail: partition-split across queues, issued from two
                    # sequencers in parallel (scalar.dma_start is HWDGE too)
                    for s in range(4):
                        pp = slice(s * 32, (s + 1) * 32)
                        eng = nc.sync if s % 2 == 0 else nc.scalar
                        eng.dma_start(
                            out=outT[ph, pp, pt0 : pt0 + 2 * TOK],
                            in_=ob[pp, :],
                        )
                else:
                    nc.sync.dma_start(
                        out=outT[ph, :, pt0 : pt0 + 2 * TOK], in_=ob[:, :]
                    )

            def emit_gate(k):
                h, t = tiles[k]
                xs = xs_full[:, h, t * TOK : (t + 1) * TOK]
                # gate psum as two SEPARATE 1-bank tiles: deps track per bank,
                # so silu_a (bank A, the down-proj overlay region) can't stall
                # the bank-B matmuls, and the down pair popped at even k waits
                # only on the early-retiring silu_a
                gpsA = gpoolA.tile([HD, 2 * TOK], F32, name=f"gpsA_{k}", tag="ga")
                gpsB = gpoolB.tile([HD, 2 * TOK], F32, name=f"gpsB_{k}", tag="gb")
                sg = sgpool.tile([HD, NCH * TOK], BF16, name=f"sg_{k}", tag="sg")
                HB = NCH * TOK // 2
                for c in range(NCH):
                    dst = gpsA if c < 2 else gpsB
                    nc.tensor.matmul(
                        dst[:, (c % 2) * TOK : (c % 2 + 1) * TOK],
                        lhsT=gw_s[:, h, c * HD : (c + 1) * HD],
                        rhs=xs,
                        start=True,
                        stop=True,
                    )
                    if c == 1:
                        nc.scalar.activation(sg[:, :HB], gpsA[:], AF.Silu)
                nc.scalar.activation(sg[:, HB:], gpsB[:], AF.Silu)
                gps_hist[k] = gpsA
                gps_hist.pop(k - 3, None)
                return gpsA, sg

            # prologue: gate+silu for tile 0
            gate_next = emit_gate(0)
            hh_pair = None
            for k in range(K):
                h, t = tiles[k]
                # pop a pending pair with a 4-tile lag: overlay into the gate
                # psum of tile k-1 (its silu retired a full iteration ago; its
                # pool slot is reallocated by gate(k+1) AFTER this emission,
                # so the pool inserts the copy->gate(k+1) dependency)
                if k % 2 == 0 and len(pend) > 1:
                    emit_down_pair(pend.pop(0), gps_hist[k - 1][:])

                gps, sg = gate_next
                ups = upool.tile([HD, NCH * TOK], F32, name=f"ups_{k}", tag="u")
                xs = xs_full[:, h, t * TOK : (t + 1) * TOK]
                for c in range(NCH):
                    nc.tensor.matmul(
                        ups[:, c * TOK : (c + 1) * TOK],
                        lhsT=uw_s[:, h, c * HD : (c + 1) * HD],
                        rhs=xs,
                        start=True,
                        stop=True,
                    )
                # next tile's gate+silu ahead of this tile's mul: PE runs it
                # during the mul; silu(k+1) overlaps mul(k) on ACT
                if k + 1 < K:
                    gate_next = emit_gate(k + 1)
                if k % 2 == 0:
                    hh_pair = hpool.tile(
                        [HD, NCH, 2 * TOK], BF16, name=f"hh_{k}", tag="hh"
                    )
                half = hh_pair[:, :, (k % 2) * TOK : (k % 2 + 1) * TOK]
                nc.vector.tensor_mul(
                    half,
                    sg[:].rearrange("p (c n) -> p c n", c=NCH),
                    ups[:].rearrange("p (c n) -> p c n", c=NCH),
                )

                if k % 2 == 1:
                    pend.append((hh_pair, h, t))
            # epilogue: two pairs remain; overlay into the two most recent
            # gate-psum slots (one per parity -> distinct banks, no serialize)
            emit_down_pair(pend.pop(0), gps_hist[K - 2][:])
            emit_down_pair(pend.pop(0), gps_hist[K - 1][:])
    nc.compile()
    return nc


def _shard_inputs(inputs):
    import ml_dtypes

    bf16 = ml_dtypes.bfloat16
    x = np.asarray(inputs["x"], dtype=np.float32)
    gw = np.asarray(inputs["gate_w"], dtype=np.float32).astype(bf16)
    uw = np.asarray(inputs["up_w"], dtype=np.float32).astype(bf16)
    dw = np.asarray(inputs["down_w"], dtype=np.float32).astype(bf16)

    xh = x.reshape(T, HEADS, HD)
    xt = np.ascontiguousarray(xh.transpose(1, 2, 0)).astype(bf16)  # [16, 128, T]

    in_maps = []
    for c in range(N_CORES):
        hs = slice(HPC * c, HPC * (c + 1))
        in_maps.append(
            {
                "xT": xt[hs],
                "gw": gw[hs],
                "uw": uw[hs],
                "dw": dw[hs],
            }
        )
    return in_maps


def run(inputs, trace=False, **spmd_kwargs):
    nc = _build_nc()
    in_maps = _shard_inputs(inputs)
    res = run_bass_kernel_spmd(
        nc, in_maps, core_ids=list(range(N_CORES)), trace=trace, **spmd_kwargs
    )
    outT = np.empty((HEADS, HD, T), dtype=np.float32)
    for c in range(N_CORES):
        outT[HPC * c : HPC * (c + 1)] = np.asarray(
            res.results[c]["outT"], dtype=np.float32
        )
    out = np.ascontiguousarray(outT.transpose(2, 0, 1)).reshape(B, S, EMB)
    return out, res


def kernel(**inputs):
    out, _ = run(inputs)
    return out
